# revision 7
# baseline (speedup 1.0000x reference)
"""Trainium2 bass kernel v3 for the GNN message-passing problem.

kernel(**inputs) -> np.ndarray [100000, 1]

Architecture (per core, edges split evenly across N_CORES cores,
dispatched as INDEPENDENT per-device jits so device execution overlaps):

Per 2048-edge chunk, per direction d (j-direction reuses x with negated
v columns via a negated stationary):
  L0: z0 = W0' @ x5   -- x5 = [r, vx, vy, vz, 1] (bias folded, K=5);
      4 row-tiles (tile_position=(32q,0)) run CONCURRENTLY in the PE
      array (K=5 <= 32), one 512-col sub-block each.
  h0 = silu(z0)       -- Act engine, native Silu, zero bias
  L1: z1 = W1 @ h0    -- 4x 512-col matmuls, K=M=128
  h1 = silu(z1 + b1)  -- Act engine
  L2: y[t] = w2 . h1  -- stationary w2emb[t] = w2 embedded in col t of
      a [128,128] zero matrix deposits sub-block-dir t's y into
      partition t of an accumulating [128,512] region (bank 0 of the
      z1_d1 tile, free after its silu; f32r matmul PSUM writes must
      start at partition 0). Drained per chunk by DVE, then DMA.

Host does the index gather (v[i]-v[j]), feature packing, and the
segment-mean + division (index-dependent scatter).
"""
import sys
sys.path.insert(0, "/opt/trn_rl_repo")
import threading
import numpy as np

N_NODES = 100000
E_TOTAL = 1600000
HSM = 3.0
N_CORES = 8
CHUNK = 2048
SB = CHUNK // 512          # sub-blocks per chunk: 4
YR = 8                     # y rows per chunk (t = d*4+q in partition t)
Epc = E_TOTAL // N_CORES
NCH = -(-Epc // CHUNK)     # chunks per core
K4 = NCH * 512             # xh columns per core
XSC = 8                    # chunks per x super-tile
WBC = 1536                 # weight blob columns


def _coalesce_and_split_waits(nc, max_waits=1):
    """Merge same-semaphore waits (keep max target), then move any
    remaining extra waits onto preceding single-wait InstNoOps (this
    walrus build rejects >1 sync-wait per instruction)."""
    import concourse.mybir as mybir
    n_split = 0
    for f in nc.m.functions:
        for blk in f.blocks:
            out = []
            for ins in blk.instructions:
                si = ins.sync_info
                if si is not None and si.on_wait and len(si.on_wait) > 1:
                    merged = {}
                    for w in si.on_wait:
                        key = (w.id, w.ant_name, str(w.sync_type),
                               str(w.wait_mode))
                        prev = merged.get(key)
                        if prev is None or w.wait_value > prev.wait_value:
                            merged[key] = w
                    waits = list(merged.values())
                    if len(waits) > max_waits:
                        for k, w in enumerate(waits[:-max_waits]):
                            nop = mybir.InstNoOp(name=f"{ins.name}-ws{k}")
                            nop.engine = ins.engine
                            nop.sync_info = mybir.SyncInfo(
                                on_wait=[w], on_update=[])
                            out.append(nop)
                            n_split += 1
                        waits = waits[-max_waits:]
                    si.on_wait = waits
                out.append(ins)
            blk.instructions = out
    return n_split


def _build_kernel(nch=NCH):
    import concourse.bass as bass
    import concourse.mybir as mybir
    from concourse.tile import TileContext

    F32 = mybir.dt.float32
    F32R = mybir.dt.float32r
    AF = mybir.ActivationFunctionType

    k4 = nch * 512
    nc = bass.Bass()
    x_d = nc.declare_dram_parameter("x", [20, k4], F32R, isOutput=False)
    wb_d = nc.declare_dram_parameter("wblob", [128, WBC], F32R,
                                     isOutput=False)
    y_d = nc.declare_dram_parameter("y", [nch * YR, 512], F32, isOutput=True)

    w1_d = wb_d[:, 256:384]
    w2e_d = wb_d[:, 384:1408]
    b1_d = wb_d[:, 1408:1409].bitcast(F32)
    bz_d = wb_d[:, 1409:1410].bitcast(F32)

    with TileContext(nc) as tc:
        with (
            tc.tile_pool(name="const", bufs=1) as cpool,
            tc.tile_pool(name="xp", bufs=2) as xp,
            tc.tile_pool(name="hp", bufs=6) as hp,
            tc.tile_pool(name="zp", bufs=2, space="PSUM") as zp,
        ):
            w0t = cpool.tile([128, 256], F32R, tag="w0")
            w1t = cpool.tile([128, 128], F32R, tag="w1")
            w2t = cpool.tile([128, 1024], F32R, tag="w2")
            b1t = cpool.tile([128, 1], F32, tag="b1")
            bzt = cpool.tile([128, 1], F32, tag="bz")
            nc.sync.dma_start(out=w0t[:], in_=wb_d[:, 0:256])
            nc.sync.dma_start(out=w1t[:], in_=w1_d)
            nc.sync.dma_start(out=w2t[:], in_=w2e_d)
            nc.sync.dma_start(out=b1t[:], in_=b1_d)
            nc.sync.dma_start(out=bzt[:], in_=bz_d)

            xt = None
            for c in range(nch):
                if c % XSC == 0:
                    xt = xp.tile([128, XSC * 512], F32R, tag="x")
                    xw = min(XSC * 512, k4 - c * 512)
                    for q in range(4):
                        nc.sync.dma_start(
                            out=xt[32 * q:32 * q + 5, 0:xw],
                            in_=x_d[5 * q:5 * q + 5,
                                    c * 512:c * 512 + xw])
                xo = (c % XSC) * 512
                # L0: row-tiled, both directions
                z0 = {}
                for d in (0, 1):
                    z0[d] = zp.tile([128, CHUNK], F32, tag="z",
                                    name=f"z0_{d}")
                    for q in range(SB):
                        nc.tensor.matmul(
                            out=z0[d][:, q * 512:(q + 1) * 512],
                            lhsT=w0t[32 * q:32 * q + 5,
                                     d * 128:(d + 1) * 128],
                            rhs=xt[32 * q:32 * q + 5, xo:xo + 512],
                            start=True, stop=True,
                            tile_position=(32 * q, 0),
                            skip_group_check=True)
                h0 = {}
                for d in (0, 1):
                    h0[d] = hp.tile([128, CHUNK], F32R, tag="h",
                                    name=f"h0_{d}")
                    nc.scalar.activation(out=h0[d][:], in_=z0[d][:],
                                         func=AF.Silu, bias=bzt[:])
                z1 = {}
                for d in (0, 1):
                    z1[d] = zp.tile([128, CHUNK], F32, tag="z",
                                    name=f"z1_{d}")
                    for q in range(SB):
                        nc.tensor.matmul(
                            out=z1[d][:, q * 512:(q + 1) * 512],
                            lhsT=w1t[:],
                            rhs=h0[d][:, q * 512:(q + 1) * 512],
                            start=True, stop=True)
                h1 = {}
                for d in (0, 1):
                    h1[d] = hp.tile([128, CHUNK], F32R, tag="h",
                                    name=f"h1_{d}")
                    nc.scalar.activation(out=h1[d][:], in_=z1[d][:],
                                         func=AF.Silu, bias=b1t[:])
                # L2: w2emb[t] deposits sub-block-dir t's y into
                # partition t of an accumulating [128,512] region (bank 0
                # of z1[1], free after its silu; f32r matmuls must write
                # PSUM from partition 0)
                for t in range(8):
                    d, q = t // 4, t % 4
                    nc.tensor.matmul(
                        out=z1[1][:, 0:512],
                        lhsT=w2t[:, t * 128:(t + 1) * 128],
                        rhs=h1[d][:, q * 512:(q + 1) * 512],
                        start=(t == 0), stop=(t == 7),
                        skip_group_check=True)
                ysb = hp.tile([YR, 512], F32, tag="ysb", name="ysb")
                nc.vector.tensor_copy(out=ysb[:], in_=z1[1][0:YR, 0:512])
                nc.sync.dma_start(
                    out=y_d[c * YR:(c + 1) * YR, :], in_=ysb[:])
    return nc


def prepare(v, r_ij, W0, b0, W1, b1, W2, b2, edge_index, n_cores=N_CORES,
            epc=None):
    """Host prep: returns (nc, in_maps, postprocess_fn)."""
    v = np.asarray(v, np.float32)
    r_ij = np.asarray(r_ij, np.float32)
    ei = np.asarray(edge_index)
    i_all = ei[0].astype(np.int64)
    j_all = ei[1].astype(np.int64)

    W0 = np.asarray(W0, np.float32)          # [128, 4]
    b0 = np.asarray(b0, np.float32).reshape(128)
    W1 = np.asarray(W1, np.float32)
    b1v = np.asarray(b1, np.float32).reshape(128)
    w2col = np.asarray(W2, np.float32).reshape(128)
    b2val = float(np.asarray(b2).reshape(()))

    w0p = np.concatenate([W0.T, b0[None, :]], axis=0)   # [5, 128]
    w0n = w0p.copy()
    w0n[1:4, :] *= -1.0

    wblob = np.zeros((128, WBC), np.float32)
    for q in range(4):
        wblob[32 * q:32 * q + 5, 0:128] = w0p
        wblob[32 * q:32 * q + 5, 128:256] = w0n
    wblob[:, 256:384] = W1.T
    w2e = np.zeros((128, 8, 128), np.float32)
    w2e[:, np.arange(8), np.arange(8)] = w2col[:, None]
    wblob[:, 384:1408] = w2e.reshape(128, 1024)
    wblob[:, 1408] = b1v
    # col 1409 stays zero (bias for L0 silus)

    epc = epc or (E_TOTAL // n_cores)
    nch = -(-epc // CHUNK)
    k4 = nch * 512

    vij_all = v[i_all] - v[j_all]
    r_all = np.sqrt((r_ij * r_ij).sum(1)) * np.float32(1.0 / HSM)
    E = i_all.shape[0]

    in_maps = []
    for c in range(n_cores):
        lo = c * epc
        hi = min(lo + epc, E)
        n = hi - lo
        feat = np.zeros((5, nch * CHUNK), np.float32)
        feat[0, :n] = r_all[lo:hi]
        feat[1:4, :n] = vij_all[lo:hi].T
        feat[4, :n] = 1.0
        # xh[5q+f, c*512 + u] = feat[f, c*2048 + q*512 + u]
        xh = (
            feat.reshape(5, nch, 4, 512)
            .transpose(2, 0, 1, 3)
            .reshape(20, k4)
        )
        in_maps.append({"x": np.ascontiguousarray(xh), "wblob": wblob})

    nc = _build_kernel(nch)
    _coalesce_and_split_waits(nc)

    def post(results):
        S_i = np.zeros(N_NODES, np.float64)
        S_j = np.zeros(N_NODES, np.float64)
        c_i = np.bincount(i_all, minlength=N_NODES)
        c_j = np.bincount(j_all, minlength=N_NODES)
        for c in range(n_cores):
            lo = c * epc
            hi = min(lo + epc, E)
            n = hi - lo
            Y = np.asarray(results[c]["y"])          # [nch*YR, 512]
            md = Y.reshape(nch, 2, 4 * 512)
            mi = md[:, 0, :].reshape(-1)[:n]
            mj = md[:, 1, :].reshape(-1)[:n]
            np.add.at(S_i, i_all[lo:hi], mi)
            np.add.at(S_j, j_all[lo:hi], mj)
        S = (S_i / np.maximum(c_i, 1) + b2val * (c_i > 0)
             + S_j / np.maximum(c_j, 1) + b2val * (c_j > 0))
        return S[:, None].astype(np.float32)

    return nc, in_maps, post


class PerDeviceRunner:
    """One independent jitted executable per device; dispatch overlaps."""

    def __init__(self, nc, n_cores):
        import jax
        import concourse.mybir as mybir
        from concourse import bass2jax

        bass2jax.install_neuronx_cc_hook()
        self.jax = jax
        self.n_cores = n_cores
        self.devices = jax.devices()[:n_cores]
        partition_name = (
            nc.partition_id_tensor.name if nc.partition_id_tensor else None
        )
        in_names, out_names, out_avals, zero_outs = [], [], [], []
        in_specs = []
        for alloc in nc.m.functions[0].allocations:
            if not isinstance(alloc, mybir.MemoryLocationSet):
                continue
            name = alloc.memorylocations[0].name
            if alloc.kind == "ExternalInput":
                if name != partition_name:
                    in_names.append(name)
                    in_specs.append(
                        (tuple(alloc.tensor_shape), mybir.dt.np(alloc.dtype))
                    )
            elif alloc.kind == "ExternalOutput":
                shape = tuple(alloc.tensor_shape)
                dtype = mybir.dt.np(alloc.dtype)
                out_names.append(name)
                out_avals.append(jax.core.ShapedArray(shape, dtype))
                zero_outs.append(np.zeros(shape, dtype))
        self.in_names = in_names
        self.out_names = out_names
        self.zero_outs = zero_outs
        n_params = len(in_names)
        self.n_params = n_params
        all_names = list(in_names) + out_names
        if partition_name is not None:
            all_names.append(partition_name)
        # No donation: the kernel writes every element of y, so the
        # zero "output" operands are never actually read -- one resident
        # buffer per device is passed (not consumed) on every call.
        donate = ()

        def _body(*args):
            operands = list(args)
            if partition_name is not None:
                operands.append(bass2jax.partition_id_tensor())
            outs = bass2jax._bass_exec_p.bind(
                *operands,
                out_avals=tuple(out_avals),
                in_names=tuple(all_names),
                out_names=tuple(out_names),
                lowering_input_output_aliases=(),
                sim_require_finite=True,
                sim_require_nnan=True,
                nc=nc,
            )
            return tuple(outs)

        # AOT-compile per device under fast_dispatch (bass_effect
        # suppressed -> JAX C++ fast dispatch path; the effectful path
        # costs ~300us of GIL-bound python per call through the tunnel).
        arg_specs = list(in_specs) + [(z.shape, z.dtype) for z in zero_outs]
        from jax.sharding import SingleDeviceSharding

        self.fns = []
        for dev in self.devices:
            def compile_fn(dev=dev):
                jitted = jax.jit(
                    _body, donate_argnums=donate, keep_unused=True
                )
                avals = [
                    jax.ShapeDtypeStruct(
                        s, dt, sharding=SingleDeviceSharding(dev)
                    )
                    for s, dt in arg_specs
                ]
                return jitted.lower(*avals).compile()

            try:
                fn = bass2jax.fast_dispatch_compile(compile_fn)
            except Exception:
                fn = jax.jit(_body, donate_argnums=donate, keep_unused=True)
            self.fns.append(fn)
        self._pool = None
        self._staged = None

    def put_inputs(self, in_maps):
        """Upload per-core inputs; returns device-resident base args."""
        jax = self.jax
        return [
            [
                jax.device_put(np.asarray(in_maps[c][n]), self.devices[c])
                for n in self.in_names
            ]
            for c in range(self.n_cores)
        ]

    def stage_outputs(self):
        """Resident zero output operands (reused: nothing is donated)."""
        if self._staged is None:
            jax = self.jax
            self._staged = [
                [
                    jax.device_put(z.copy(), self.devices[c])
                    for z in self.zero_outs
                ]
                for c in range(self.n_cores)
            ]
        return self._staged

    def dispatch(self, base, outs):
        """Fire all devices; fast-dispatch (C++ path) calls enqueue
        asynchronously in ~40us each, so a plain loop overlaps all
        device executions (a thread pool only adds GIL overhead here)."""
        return [
            self.fns[c](*base[c], *outs[c]) for c in range(self.n_cores)
        ]

    def run(self, in_maps):
        jax = self.jax
        base = self.put_inputs(in_maps)
        res = self.dispatch(base, self.stage_outputs())
        jax.block_until_ready(res)
        return [
            {n: np.asarray(res[c][i]) for i, n in enumerate(self.out_names)}
            for c in range(self.n_cores)
        ]


def kernel(v, r_ij, W0, b0, W1, b1, W2, b2, edge_index):
    nc, in_maps, post = prepare(v, r_ij, W0, b0, W1, b1, W2, b2, edge_index)
    r = PerDeviceRunner(nc, N_CORES)
    return post(r.run(in_maps))
